# revision 9
# baseline (speedup 1.0000x reference)
"""Bidirectional Elman RNN + MLP head on 8 Trainium2 NeuronCores (Bass/Tile).

Problem: secuencia [512, 256, 300] f32; two independent 512-step Elman scans
(forward / time-reversed), h' = tanh(x@Wx + h@Wh + b), H=256; concat final
hidden states -> MLP head -> tanh -> [256].

Sharding: direction-split data parallel. Cores 0-3 run the forward scan on
batch shards of 64; cores 4-7 run the backward scan on the same shards
(their input is host-time-reversed so the SPMD program is identical).
Params are replicated per direction. The tiny MLP head runs as a second
launch, batch-sharded 8 ways.

Per-core layout (launch 1):
  - x is staged host-side as x.T [300, 512*64] bf16 (col = t*64 + b).
  - Phase A (pipelined with B): Xproj[t] = x_t @ Wx + b computed as
    weight-stationary matmuls in 8-timestep blocks (N=512 columns), PSUM ->
    SBUF copy w/ bias via ScalarE (m=0) and VectorE (m=1), stored bf16 as
    xq[p, t, m*64+b] = Xproj[t, b, m*128+p].
  - Phase B: 512 sequential steps. Per step: identity-weight matmul injects
    xq[t] into a PSUM bank (start=True), 4 accumulating matmuls add
    Wh.T @ h (2 K-chunks x 2 M-chunks), one ScalarE tanh PSUM->SBUF bf16
    produces h_{t+1} in transposed layout h[p, m*64+b]. Xproj matmuls of
    later blocks are interleaved between steps to fill PE wait bubbles.
"""

import os
import sys

import numpy as np
import ml_dtypes

for _p in ("/opt/trn_rl_repo",):
    if os.path.isdir(_p) and _p not in sys.path:
        sys.path.append(_p)

import concourse.bass as bass  # noqa: E402
import concourse.mybir as mybir  # noqa: E402
import concourse.tile as tile  # noqa: E402
from concourse import bacc  # noqa: E402
from concourse.bass_utils import run_bass_kernel_spmd  # noqa: E402

BF16 = ml_dtypes.bfloat16
F32 = np.float32

SEQ, B, IN, H = 512, 256, 300, 256
NCORES = 8
BPC = B // (NCORES // 2)  # 64: batch per core (each shard done by 1 fwd + 1 bwd core)
TBLK = 8  # timesteps per xproj block (8 * 64 = 512 moving columns)
KCH_IN = [(0, 128), (128, 128), (256, 44)]  # K chunks of IN=300
HB = BPC  # 64 columns per m-half in the h layout

# module-level knobs for the test harness
TRACE = False
TRACE_KWARGS = {}
LAST = {}


OPT = {
    "psr_bufs": 4,
    "psx_bufs": 3,
    "h_bufs": 3,
    "inject": "ident",  # 'ident' | 'dve'
    "tanh_split": 1,  # 1 or 2
    "drain_per_step": 1,
}


def build_launch1(seq=SEQ, opt=None):
    """One direction's scan for a 64-batch shard. SPMD across all 8 cores."""
    cfg = dict(OPT)
    if opt:
        cfg.update(opt)
    nblk = seq // TBLK
    nc = bacc.Bacc("TRN2", target_bir_lowering=False, debug=False, num_devices=NCORES)
    dt = mybir.dt

    xt_d = nc.dram_tensor("xt", [IN, seq * BPC], dt.bfloat16, kind="ExternalInput")
    wx_d = nc.dram_tensor("wx", [IN, H], dt.bfloat16, kind="ExternalInput")
    wh_d = nc.dram_tensor("wh", [H, H], dt.bfloat16, kind="ExternalInput")
    bv_d = nc.dram_tensor("bv", [128, 2], dt.float32, kind="ExternalInput")
    id_d = nc.dram_tensor("ident", [128, 128], dt.bfloat16, kind="ExternalInput")
    ho_d = nc.dram_tensor("ho", [128, 2 * HB], dt.float32, kind="ExternalOutput")

    with tile.TileContext(nc) as tc:
        with (
            tc.tile_pool(name="wpool", bufs=1) as wpool,
            tc.tile_pool(name="xpool", bufs=3) as xpool,
            tc.tile_pool(name="xqpool", bufs=nblk) as xqpool,
            tc.tile_pool(name="hpool", bufs=cfg["h_bufs"]) as hpool,
            tc.tile_pool(name="opool", bufs=1) as opool,
            tc.tile_pool(name="psx", bufs=cfg["psx_bufs"], space="PSUM") as psxpool,
            tc.tile_pool(name="psr", bufs=cfg["psr_bufs"], space="PSUM") as psrpool,
        ):
            # ---- weights / constants ----
            wxt = wpool.tile([128, 3, H], dt.bfloat16)
            for c, (o, k) in enumerate(KCH_IN):
                nc.sync.dma_start(wxt[0:k, c, :], wx_d.ap()[o : o + k, :])
            wht = wpool.tile([128, 2, H], dt.bfloat16)
            for c in range(2):
                nc.sync.dma_start(wht[:, c, :], wh_d.ap()[c * 128 : (c + 1) * 128, :])
            bvt = wpool.tile([128, 2], dt.float32)
            nc.sync.dma_start(bvt[:], bv_d.ap()[:])
            idt = wpool.tile([128, 128], dt.bfloat16)
            nc.sync.dma_start(idt[:], id_d.ap()[:])

            if cfg["tanh_split"] == 2:
                h_prev = []
                for m in range(2):
                    h0m = hpool.tile([128, HB], dt.bfloat16, name=f"h0_{m}", tag=f"h{m}")
                    nc.gpsimd.memset(h0m[:], 0.0)
                    h_prev.append(h0m)
            else:
                h_prev = hpool.tile([128, 2 * HB], dt.bfloat16, name="h0")
                nc.gpsimd.memset(h_prev[:], 0.0)

            xq_tiles = []
            pending = []  # deferred xproj matmul thunks, interleaved into steps

            def emit_xproj_block(blk):
                xsb = xpool.tile([128, 3, TBLK * BPC], dt.bfloat16, name="xsb")
                for c, (o, k) in enumerate(KCH_IN):
                    nc.sync.dma_start(
                        xsb[0:k, c, :],
                        xt_d.ap()[o : o + k, blk * TBLK * BPC : (blk + 1) * TBLK * BPC],
                    )
                xq = xqpool.tile([128, TBLK, 2 * HB], dt.bfloat16, name="xq")
                xq_tiles.append(xq)
                for m in range(2):
                    psx = psxpool.tile([128, TBLK, BPC], dt.float32, name="psx")

                    def mk(c, k, m, psx, xsb, xq):
                        def go():
                            nc.tensor.matmul(
                                psx[:],
                                wxt[0:k, c, m * 128 : (m + 1) * 128],
                                xsb[0:k, c, :],
                                start=(c == 0),
                                stop=(c == 2),
                            )
                            if c == 2:
                                if m == 0:
                                    nc.scalar.activation(
                                        xq[:, :, 0:HB],
                                        psx[:],
                                        mybir.ActivationFunctionType.Identity,
                                        bias=bvt[:, 0:1],
                                    )
                                else:
                                    nc.vector.tensor_scalar_add(
                                        xq[:, :, HB : 2 * HB], psx[:], bvt[:, 1:2]
                                    )

                        return go

                    for c, (_, k) in enumerate(KCH_IN):
                        pending.append(mk(c, k, m, psx, xsb, xq))

            def drain_one():
                if pending:
                    pending.pop(0)()

            def emit_step(t, last):
                nonlocal h_prev
                xq = xq_tiles[t // TBLK]
                ti = t % TBLK
                ident_inject = cfg["inject"] == "ident"
                split2 = cfg["tanh_split"] == 2
                odt = dt.float32 if last else dt.bfloat16
                if split2:
                    # two independent half-chains: psum bank + h tile per m-half
                    h_new = [None, None]
                    for m in range(2):
                        psr = psrpool.tile([128, HB], dt.float32, name=f"psr{m}", tag=f"psr{m}")
                        nc.tensor.matmul(
                            psr[:],
                            idt[:],
                            xq[:, ti, m * HB : (m + 1) * HB],
                            start=True,
                            stop=False,
                        )
                        if m == 0:
                            drain_one()
                        for c in range(2):
                            nc.tensor.matmul(
                                psr[:],
                                wht[:, c, m * 128 : (m + 1) * 128],
                                h_prev[c][:] if isinstance(h_prev, list) else h_prev[:, c * HB : (c + 1) * HB],
                                start=False,
                                stop=(c == 1),
                            )
                        if last:
                            hn = opool.tile([128, HB], odt, name=f"hf{m}")
                        else:
                            hn = hpool.tile([128, HB], odt, name=f"h{m}", tag=f"h{m}")
                        nc.scalar.activation(
                            hn[:], psr[:], mybir.ActivationFunctionType.Tanh
                        )
                        h_new[m] = hn
                    if last:
                        for m in range(2):
                            nc.sync.dma_start(
                                ho_d.ap()[:, m * HB : (m + 1) * HB], h_new[m][:]
                            )
                    else:
                        h_prev = h_new
                    return
                psr = psrpool.tile([128, 2 * HB], dt.float32, name="psr")
                if ident_inject:
                    nc.tensor.matmul(
                        psr[:], idt[:], xq[:, ti, :], start=True, stop=False
                    )
                for _ in range(cfg["drain_per_step"]):
                    drain_one()
                for c in range(2):
                    for m in range(2):
                        nc.tensor.matmul(
                            psr[:, m * HB : (m + 1) * HB],
                            wht[:, c, m * 128 : (m + 1) * 128],
                            h_prev[:, c * HB : (c + 1) * HB],
                            start=(not ident_inject and c == 0),
                            stop=(c == 1 and m == 1),
                        )
                # tanh input: psum directly (ident inject) or psum+xq via DVE
                if ident_inject:
                    tin = psr
                else:
                    v = hpool.tile([128, 2 * HB], dt.bfloat16, name="v", tag="v")
                    nc.vector.tensor_tensor(
                        v[:], psr[:], xq[:, ti, :], mybir.AluOpType.add
                    )
                    tin = v
                if last:
                    h_new = opool.tile([128, 2 * HB], odt, name="hf")
                else:
                    h_new = hpool.tile([128, 2 * HB], odt, name="h")
                nc.scalar.activation(
                    h_new[:], tin[:], mybir.ActivationFunctionType.Tanh
                )
                if last:
                    nc.sync.dma_start(ho_d.ap()[:], h_new[:])
                else:
                    h_prev = h_new

            # prologue: 2 blocks of xproj before the scan starts
            emit_xproj_block(0)
            for _ in range(6):
                drain_one()
            emit_xproj_block(1)
            for blk in range(2, nblk + 2):
                if blk < nblk:
                    emit_xproj_block(blk)
                t0 = (blk - 2) * TBLK
                for i in range(TBLK):
                    t = t0 + i
                    emit_step(t, last=(t == seq - 1))

    nc.compile()
    return nc


def build_launch2():
    """MLP head, batch-sharded: each core does 32 rows of the 256-batch head."""
    BS = B // NCORES  # 32
    nc = bacc.Bacc("TRN2", target_bir_lowering=False, debug=False, num_devices=NCORES)
    dt = mybir.dt

    hc_d = nc.dram_tensor("hc", [128, 4, BS], dt.bfloat16, kind="ExternalInput")
    f1_d = nc.dram_tensor("f1", [2 * H, 2 * H], dt.bfloat16, kind="ExternalInput")
    f2_d = nc.dram_tensor("f2", [2 * H, H], dt.bfloat16, kind="ExternalInput")
    fs_d = nc.dram_tensor("fs", [128, 2], dt.bfloat16, kind="ExternalInput")
    b1_d = nc.dram_tensor("hb1", [128, 4], dt.float32, kind="ExternalInput")
    b2_d = nc.dram_tensor("hb2", [128, 2], dt.float32, kind="ExternalInput")
    b3_d = nc.dram_tensor("hb3", [1, 1], dt.float32, kind="ExternalInput")
    o_d = nc.dram_tensor("out", [1, BS], dt.float32, kind="ExternalOutput")

    with tile.TileContext(nc) as tc:
        with (
            tc.tile_pool(name="sb", bufs=1) as sb,
            tc.tile_pool(name="ps", bufs=2, space="PSUM") as ps,
        ):
            hct = sb.tile([128, 4, BS], dt.bfloat16)
            nc.sync.dma_start(hct[:], hc_d.ap()[:])
            f1t = sb.tile([128, 4, 2 * H], dt.bfloat16)
            for c in range(4):
                nc.sync.dma_start(f1t[:, c, :], f1_d.ap()[c * 128 : (c + 1) * 128, :])
            f2t = sb.tile([128, 4, H], dt.bfloat16)
            for c in range(4):
                nc.sync.dma_start(f2t[:, c, :], f2_d.ap()[c * 128 : (c + 1) * 128, :])
            fst = sb.tile([128, 2], dt.bfloat16)
            nc.sync.dma_start(fst[:], fs_d.ap()[:])
            b1t = sb.tile([128, 4], dt.float32)
            nc.sync.dma_start(b1t[:], b1_d.ap()[:])
            b2t = sb.tile([128, 2], dt.float32)
            nc.sync.dma_start(b2t[:], b2_d.ap()[:])
            b3t = sb.tile([1, 1], dt.float32)
            nc.sync.dma_start(b3t[:], b3_d.ap()[:])

            a1 = sb.tile([128, 4, BS], dt.bfloat16)
            for m in range(4):
                p1 = ps.tile([128, BS], dt.float32, name="p1")
                for c in range(4):
                    nc.tensor.matmul(
                        p1[:],
                        f1t[:, c, m * 128 : (m + 1) * 128],
                        hct[:, c, :],
                        start=(c == 0),
                        stop=(c == 3),
                    )
                nc.scalar.activation(
                    a1[:, m, :],
                    p1[:],
                    mybir.ActivationFunctionType.Relu,
                    bias=b1t[:, m : m + 1],
                )
            a2 = sb.tile([128, 2, BS], dt.bfloat16)
            for m in range(2):
                p2 = ps.tile([128, BS], dt.float32, name="p2")
                for c in range(4):
                    nc.tensor.matmul(
                        p2[:],
                        f2t[:, c, m * 128 : (m + 1) * 128],
                        a1[:, c, :],
                        start=(c == 0),
                        stop=(c == 3),
                    )
                nc.scalar.activation(
                    a2[:, m, :],
                    p2[:],
                    mybir.ActivationFunctionType.Relu,
                    bias=b2t[:, m : m + 1],
                )
            p3 = ps.tile([1, BS], dt.float32, name="p3")
            for c in range(2):
                nc.tensor.matmul(
                    p3[:], fst[:, c : c + 1], a2[:, c, :], start=(c == 0), stop=(c == 1)
                )
            ot = sb.tile([1, BS], dt.float32)
            nc.scalar.activation(
                ot[:], p3[:], mybir.ActivationFunctionType.Tanh, bias=b3t[:, 0:1]
            )
            nc.sync.dma_start(o_d.ap()[:], ot[:])

    nc.compile()
    return nc


_BUILD_CACHE = {}


def _get(name, fn):
    if name not in _BUILD_CACHE:
        _BUILD_CACHE[name] = fn()
    return _BUILD_CACHE[name]


def _prep_launch1_inputs(secuencia, W1x, W1h, b1, W2x, W2h, b2):
    """Per-core in_maps for launch 1."""
    ident = np.eye(128, dtype=BF16)
    packs = []
    for d, (Wx, Wh, bb) in enumerate([(W1x, W1h, b1), (W2x, W2h, b2)]):
        wx = np.ascontiguousarray(Wx).astype(BF16)
        wh = np.ascontiguousarray(Wh).astype(BF16)
        bv = np.ascontiguousarray(np.asarray(bb, F32).reshape(2, 128).T)  # [128,2]
        packs.append((wx, wh, bv))
    in_maps = []
    for core in range(NCORES):
        d = core // 4  # 0: fwd, 1: bwd
        s = core % 4
        xs = secuencia[:, s * BPC : (s + 1) * BPC, :]
        if d == 1:
            xs = xs[::-1]
        # [SEQ, BPC, IN] -> [IN, SEQ*BPC], col = t*BPC + b
        xt = np.ascontiguousarray(xs.transpose(2, 0, 1).reshape(IN, SEQ * BPC)).astype(
            BF16
        )
        wx, wh, bv = packs[d]
        in_maps.append({"xt": xt, "wx": wx, "wh": wh, "bv": bv, "ident": ident})
    return in_maps


def _h_from_ho(ho):
    """[128, 2*HB] f32 -> h [BPC, 256] (h[b, m*128+p] = ho[p, m*HB+b])."""
    return ho.reshape(128, 2, HB).transpose(2, 1, 0).reshape(HB, H).astype(F32)


def kernel(
    secuencia,
    W1x,
    W1h,
    b1,
    W2x,
    W2h,
    b2,
    fc1_w,
    fc1_b,
    fc2_w,
    fc2_b,
    fs_w,
    fs_b,
):
    secuencia = np.asarray(secuencia, F32)
    nc1 = _get("l1", build_launch1)
    in_maps = _prep_launch1_inputs(
        secuencia,
        np.asarray(W1x, F32),
        np.asarray(W1h, F32),
        np.asarray(b1, F32),
        np.asarray(W2x, F32),
        np.asarray(W2h, F32),
        np.asarray(b2, F32),
    )
    res1 = run_bass_kernel_spmd(
        nc1,
        in_maps,
        core_ids=list(range(NCORES)),
        trace=TRACE,
        **TRACE_KWARGS,
    )
    LAST["res1"] = res1
    h1 = np.concatenate(
        [_h_from_ho(res1.results[c]["ho"]) for c in range(4)], axis=0
    )  # [256, 256]
    h2 = np.concatenate([_h_from_ho(res1.results[c]["ho"]) for c in range(4, 8)], axis=0)
    hc = np.concatenate([h1, h2], axis=1)  # [256, 512]

    # ---- launch 2: head ----
    nc2 = _get("l2", build_launch2)
    BS = B // NCORES
    hcT = hc.T.astype(BF16)  # [512, 256]
    f1 = np.ascontiguousarray(np.asarray(fc1_w, F32)).astype(BF16)
    f2 = np.ascontiguousarray(np.asarray(fc2_w, F32)).astype(BF16)
    fs = np.ascontiguousarray(np.asarray(fs_w, F32).reshape(2, 128).T).astype(BF16)
    hb1 = np.ascontiguousarray(np.asarray(fc1_b, F32).reshape(4, 128).T)
    hb2 = np.ascontiguousarray(np.asarray(fc2_b, F32).reshape(2, 128).T)
    hb3 = np.asarray(fs_b, F32).reshape(1, 1)
    in_maps2 = []
    for core in range(NCORES):
        cols = slice(core * BS, (core + 1) * BS)
        hct = np.ascontiguousarray(hcT[:, cols].reshape(4, 128, BS).transpose(1, 0, 2))
        in_maps2.append(
            {
                "hc": hct.astype(BF16),
                "f1": f1,
                "f2": f2,
                "fs": fs,
                "hb1": hb1,
                "hb2": hb2,
                "hb3": hb3,
            }
        )
    res2 = run_bass_kernel_spmd(
        nc2, in_maps2, core_ids=list(range(NCORES)), trace=TRACE, **TRACE_KWARGS
    )
    LAST["res2"] = res2
    out = np.concatenate([res2.results[c]["out"][0] for c in range(NCORES)])
    return out.astype(F32)
